# revision 56
# baseline (speedup 1.0000x reference)
"""Expert-parallel MoE FFN kernel for Trainium2 (Bass/Tile).

Problem: per-expert grouped-GEMM FFN
    y[e] = relu(x[e] @ wi[e]) @ wo[e]
with E=8 experts, x:[E,4096,1024] fp32, wi:[E,1024,4096], wo:[E,4096,1024].
Output: [E*4096, 1024] fp32.

Sharding: expert dim E across the 8 NeuronCores (1 expert per core, no
cross-core communication). Each core runs the same SPMD program on its
expert's slabs.

Strategy: the PE instruction stream is pure GEMM matmuls (4096 of them);
everything else is arranged around keeping it issue-bound at ~216 ns per
512-col bf16 matmul (1 cycle/row at 2.4 GHz; ~885 us floor; measured
~904-906 us end to end with the stream itself at the floor, vs the
1076 us float32r baseline).
 - All operands are pre-transformed on the HOST: x transposed + cast to
   bf16, wi/wo cast to bf16, each packed in per-partition-contiguous DMA
   layouts (128 descriptor rows per chunk -> cheap descriptor generation
   and near-peak queue bandwidth). bf16 matmul runs at the same PE rate
   as float32r; end-to-end error ~3.4e-3 vs the 2e-2 budget.
 - Both weight matrices stay fully resident in SBUF (8+8 MB of 28 MB);
   total DMA is 24 MB in + 16 MB out per core, so the PE never waits on
   HBM in steady state.
 - Startup is a DMA race (aggregate ~220 GB/s across the sync/scalar/
   gpsimd queues vs mm1 eating 145 GB/s of wi from t~15us): wi is split
   into 13 graded chunks (128-col starters, 256-col round-robin, coarse
   tail) spread EDF-style over the three queues; block-0 xT is split
   three ways; wo is queued behind each engine's wi chunks so its
   transfers only use leftover bandwidth. Moving the first matmul
   earlier does NOT help: it tightens every wi deadline by the same
   amount and the queues can't cover it (measured).
 - mm1: hT[f, c] = relu(wi-tile.T @ xT) accumulated over 8 d-chunks in
   rotating PSUM banks; ReLU on the ScalarE PSUM->SBUF copy, bf16 out.
 - mm2: yT[d, c] = sum_f wo-tile[f, d].T @ hT[f, c]; d-tile-outer so each
   of the 8 d-tiles accumulates over all 32 f-chunks in one rotating
   PSUM bank and flushes (ScalarE/VectorE alternating) while the next
   d-tile computes. Host transposes yT back to y.
Note: the device occasionally drops to a ~2.0 GHz p-state (all matmuls
uniformly 1.2x slower, 454 ns vs 379 ns duration); timings quoted are
full-clock runs.
"""

import numpy as np

P = 128
E = 8
C = 4096
D_MODEL = 1024
D_FF = 4096
CB = 512  # token block

# wi DMA chunk plan: (start f-col, width f-cols, issuing engine). Graded so
# the first f-tiles land quickly; engines rotate so the three DMA queues
# stream in parallel. Sum of widths must equal D_FF.
WI_CHUNK_PLAN = [
    (0, 128, "sync"),
    (128, 128, "scalar"),
    (256, 128, "gpsimd"),
    (384, 256, "sync"),
    (640, 256, "scalar"),
    (896, 256, "gpsimd"),
    (1152, 256, "sync"),
    (1408, 256, "scalar"),
    (1664, 256, "gpsimd"),
    (1920, 512, "sync"),
    (2432, 512, "scalar"),
    (2944, 512, "gpsimd"),
    (3456, 640, "sync"),
]


def build_bass(C=C, D=D_MODEL, F=D_FF, CB=CB):
    import concourse.bacc as bacc
    import concourse.tile as tile
    from concourse import mybir

    f32 = mybir.dt.float32
    bf16 = mybir.dt.bfloat16
    relu = mybir.ActivationFunctionType.Relu

    assert C % CB == 0 and CB == 512 and D % P == 0 and F % P == 0
    NB = C // CB  # token blocks
    DCH = D // P  # d_model chunks (contraction of mm1, and d-tiles of mm2 out)
    FCH = F // P  # d_ff chunks (mm1 outputs, contraction of mm2)
    FC = 8  # wi/wo DMA chunks
    FPC = FCH // FC  # f-tiles per chunk

    nc = bacc.Bacc("TRN2", target_bir_lowering=False, debug=False)
    # Host-packed layouts: one row per SBUF partition, fully contiguous.
    # xL row p  = [b, ko, c]: x.T[ko*128+p, b*CB+c]          (bf16)
    # wiL row p = graded chunks [fc][ko][fw] (widths WI_WIDTHS f-cols)
    # woL row p = [fc, fo, d]: wo[(4*fc+fo)*128+p, d]        (bf16)
    xL = nc.dram_tensor("xL", [P, NB, DCH, CB], bf16, kind="ExternalInput").ap()
    wiL = nc.dram_tensor("wiL", [P, DCH * F], bf16, kind="ExternalInput").ap()
    woL = nc.dram_tensor("woL", [P, FC, FPC, D], bf16, kind="ExternalInput").ap()
    yT = nc.dram_tensor("yT", [D, C], f32, kind="ExternalOutput").ap()
    yT_r = yT.rearrange("(dt p) c -> p dt c", p=P)  # [128, DCH, C]

    with tile.TileContext(nc) as tc:
        with (
            tc.tile_pool(name="const", bufs=1) as const_pool,
            tc.tile_pool(name="wi", bufs=1) as wi_pool,
            tc.tile_pool(name="wo", bufs=1) as wo_pool,
            tc.tile_pool(name="ht", bufs=3) as ht_pool,
            tc.tile_pool(name="xt", bufs=2) as xt_pool,
            tc.tile_pool(name="ys", bufs=2) as ys_pool,
            tc.tile_pool(name="psum", bufs=8, space="PSUM") as psum_pool,
        ):
            warm = const_pool.tile([P, 512], bf16)
            nc.gpsimd.memset(warm[:], 0.0)

            # Weight residency. Every chunk is [128 partitions x contiguous
            # bytes]; spread across the three DMA-capable engines
            # (sync/scalar/gpsimd) so descriptor generation and queue
            # bandwidth stay ahead of mm1/mm2 consumption. wi chunks are
            # graded (small first) so mm1 can start ~14us in: each queue
            # moves ~0.1 MB/us and mm1 eats f-tiles at one per ~1.73us.
            wi_sb = wi_pool.tile([P, DCH * F], bf16)
            wo_sb = wo_pool.tile([P, FC, FPC, D], bf16)

            def wi_lhsT(f, ko):
                """AP of the [128,128] wi tile for (f-tile, ko) in the graded
                chunk packing."""
                s0, w, _ = next(
                    c for c in WI_CHUNK_PLAN if c[0] <= f * P < c[0] + c[1]
                )
                off = DCH * s0 + ko * w + (f * P - s0)
                return wi_sb[:, off : off + P]

            def issue_wi(chunks):
                for s0, w, eng in chunks:
                    getattr(nc, eng).dma_start(
                        wi_sb[:, DCH * s0 : DCH * (s0 + w)],
                        wiL[:, DCH * s0 : DCH * (s0 + w)],
                    )

            # wi chunk 0 heads the sync queue, ahead of everything else.
            issue_wi(WI_CHUNK_PLAN[:1])

            # wo chunks are appended to each engine's queue after its wi
            # chunks; queues serialize, so wo transfers only start once that
            # queue's wi share is done (~45us), leaving the early bandwidth
            # to wi. Deadlines (mm2 of block 0 starts ~72us) are loose.
            WO_ENGINES = ["gpsimd", "scalar", "gpsimd", "scalar",
                          "gpsimd", "scalar", "gpsimd", "scalar"]

            def issue_wo_chunk(fc):
                getattr(nc, WO_ENGINES[fc]).dma_start(wo_sb[:, fc], woL[:, fc])

            def ps_tile():
                return psum_pool.tile([P, CB], f32, tag="ps", name="ps")

            # Warm the PE (p-state ramp) with dependency-free matmuls while
            # the first xT/wi DMAs are still in flight; long enough that
            # real work starts just as the graded wi chunks can sustain it.
            for _ in range(5):
                pw = ps_tile()
                for w in range(4):
                    nc.tensor.matmul(
                        pw[:],
                        lhsT=warm[:, :P],
                        rhs=warm[:],
                        start=(w == 0),
                        stop=(w == 3),
                    )

            for b in range(NB):
                c0 = b * CB
                xTb = xt_pool.tile([P, DCH, CB], bf16, tag="xt", name="xTb")
                if b == 0:
                    # Block 0's xT is on the critical path: split it three
                    # ways so it lands in parallel with wi chunk 0 on sync.
                    nc.scalar.dma_start(xTb[:, :3], xL[:, 0, :3])
                    nc.gpsimd.dma_start(xTb[:, 3:6], xL[:, 0, 3:6])
                    nc.sync.dma_start(xTb[:, 6:], xL[:, 0, 6:])
                    issue_wi(WI_CHUNK_PLAN[1:])
                else:
                    nc.sync.dma_start(xTb[:], xL[:, b])

                # --- mm1: hT[f, c] = relu(x @ wi)^T for this block ---
                # hT is split into two half-tiles (f<FH and f>=FH) so the
                # pool can triple-buffer 16KB halves.
                FH = FCH // 2
                hTs = []
                for half in range(2):
                    hTh = ht_pool.tile([P, FH, CB], bf16, tag="ht", name="hTh")
                    hTs.append(hTh)
                    for fi in range(FH):
                        f = half * FH + fi
                        ph = ps_tile()
                        for ko in range(DCH):
                            nc.tensor.matmul(
                                ph[:],
                                lhsT=wi_lhsT(f, ko),
                                rhs=xTb[:, ko, :],
                                start=(ko == 0),
                                stop=(ko == DCH - 1),
                            )
                        nc.scalar.activation(hTh[:, fi, :], ph[:], relu)
                        if b == 0 and 14 <= f < 22:
                            issue_wo_chunk(f - 14)

                # --- mm2: yT[d, c] = sum_f wo[f,d]^T @ hT[f,c] ---
                for dt in range(DCH):
                    py = psum_pool.tile([P, CB], f32, tag="ps", name="py")
                    for f in range(FCH):
                        nc.tensor.matmul(
                            py[:],
                            lhsT=wo_sb[
                                :, f // FPC, f % FPC, dt * P : (dt + 1) * P
                            ],
                            rhs=hTs[f // FH][:, f % FH, :],
                            start=(f == 0),
                            stop=(f == FCH - 1),
                        )
                    ysb = ys_pool.tile([P, CB], f32, tag="ys", name="ysb")
                    if b == NB - 1 and dt == DCH - 1:
                        # Final flush is the kernel tail: split it across
                        # ScalarE+VectorE and two DMA queues.
                        H = CB // 2
                        nc.scalar.copy(ysb[:, :H], py[:, :H])
                        nc.vector.tensor_copy(ysb[:, H:], py[:, H:])
                        nc.scalar.dma_start(
                            yT_r[:, dt, c0 : c0 + H], ysb[:, :H]
                        )
                        nc.sync.dma_start(
                            yT_r[:, dt, c0 + H : c0 + CB], ysb[:, H:]
                        )
                    else:
                        if dt % 2 == 0:
                            nc.scalar.copy(ysb[:], py[:])
                        else:
                            nc.vector.tensor_copy(ysb[:], py[:])
                        nc.sync.dma_start(yT_r[:, dt, c0 : c0 + CB], ysb[:])

    nc.compile()
    return nc


_NC_CACHE = {}


def _get_nc(shape_key):
    if shape_key not in _NC_CACHE:
        _NC_CACHE[shape_key] = build_bass(*shape_key)
    return _NC_CACHE[shape_key]


def prepare_in_maps(xs, wis, wos):
    """Host-side relayout: transpose x, cast to bf16, pack per-partition
    contiguous DMA layouts (see dram tensor comments in build_bass)."""
    import ml_dtypes

    bf16 = ml_dtypes.bfloat16
    e = xs.shape[0]
    NB, DCH, FCH, FC = C // CB, D_MODEL // P, D_FF // P, 8
    FPC = FCH // FC

    # xL[p, b, ko, c] = xT[ko*128+p, b*CB+c] = x[b*CB+c, ko*128+p]
    xLa = (
        xs.reshape(e, NB, CB, DCH, P)
        .transpose(0, 4, 1, 3, 2)
        .astype(bf16)
    )  # [e, P, NB, DCH, CB]
    # wiL: graded chunks, each packed [p, ko, fw] and concatenated flat.
    wi16 = wis.astype(bf16).reshape(e, DCH, P, D_FF)  # [e, ko, p, f]
    segs = [
        np.ascontiguousarray(
            wi16[:, :, :, s0 : s0 + w].transpose(0, 2, 1, 3)
        ).reshape(e, P, DCH * w)
        for s0, w, _ in WI_CHUNK_PLAN
    ]
    wiLa = np.concatenate(segs, axis=2)  # [e, P, DCH*F]
    # woL[p, fc, fo, d] = wo[(fc*FPC+fo)*128+p, d]
    woLa = (
        wos.reshape(e, FC, FPC, P, D_MODEL)
        .transpose(0, 3, 1, 2, 4)
        .astype(bf16)
    )  # [e, P, FC, FPC, D]
    return [
        {
            "xL": np.ascontiguousarray(xLa[i]),
            "wiL": np.ascontiguousarray(wiLa[i]),
            "woL": np.ascontiguousarray(woLa[i]),
        }
        for i in range(e)
    ]


def gather_output(res, e=E):
    """Transpose each core's yT [D, C] back to y [C, D] and stack."""
    yT = np.stack([res.results[i]["yT"] for i in range(e)])  # [E, D, C]
    return (
        np.ascontiguousarray(np.transpose(yT, (0, 2, 1)))
        .reshape(-1, yT.shape[1])
        .astype(np.float32)
    )


def kernel(dispatched_states, fused_wi_weight, fused_wo_weight):
    from concourse.bass_utils import run_bass_kernel_spmd

    xs = np.asarray(dispatched_states, dtype=np.float32)
    wis = np.asarray(fused_wi_weight, dtype=np.float32)
    wos = np.asarray(fused_wo_weight, dtype=np.float32)
    e, c, d = xs.shape
    f = wis.shape[2]
    assert (e, c, d, f) == (E, C, D_MODEL, D_FF), (e, c, d, f)

    nc = _get_nc((c, d, f, CB))
    in_maps = prepare_in_maps(xs, wis, wos)
    res = run_bass_kernel_spmd(nc, in_maps, core_ids=list(range(e)))
    return gather_output(res, e)
